# revision 4
# baseline (speedup 1.0000x reference)
"""AFT attention kernel for Trainium2, SPMD across 8 NeuronCores.

Math (reference):
    q, k, v = split(qkv)                       # each (B, C, H, W)
    kn   = softmax(k over HW)
    kv   = kn * v
    w    = expm1(weights)                      # (C, 1, KS, KS)
    k_c  = dwconv(kn, w) + sum_HW(kn)          # sum_HW(kn) == 1
    kv_c = dwconv(kv, w) + sum_HW(kv)
    out  = sigmoid(q) * kv_c / (k_c + 1e-8)

Because the softmax normalizer is constant per (channel, image) and the conv is
linear, dividing numerator and denominator by it cancels the softmax entirely:
    e  = exp(k)         p  = e * v
    out = sigmoid(q) * (dwconv(p, w) + sum(p)) / (dwconv(e, w) + sum(e))
(the +1e-8 is negligible against a denominator ~= sum(e) ~ 4k).

The depthwise conv is computed on the TensorEngine as 31 Toeplitz matmuls
(contraction over h_in with banded B-matrices built host-side from the weights;
the kx taps are free-dim offset slices of width-padded image tiles) accumulated
in PSUM, with the global-sum bias folded in as one extra matmul per bank.
Four channels run concurrently on the 4 64x64 PE array quadrants.

Sharding: channels (192 / 8 cores = 24 per core), zero collectives.
"""
import numpy as np
from contextlib import ExitStack

import concourse.bass as bass
import concourse.mybir as mybir
import concourse.tile as tile
from concourse import bacc
from concourse.bass_utils import run_bass_kernel_spmd

KS = 31
PAD = KS // 2
H = W = 64
NIMG = 16
C_FULL = 192
NCORES = 8
C_CORE = C_FULL // NCORES      # 24
NGROUPS = C_CORE // 4          # 6
WPAD = W + 2 * PAD             # 94

f32 = mybir.dt.float32
bf16 = mybir.dt.bfloat16

_CACHE = {}


def _build_bmats(w_exp):
    """w_exp: (C, KS, KS) f32 -> per-core B tiles.

    Returns (NCORES, NGROUPS, 128, 2, KS, H) f32 where
    [r, g, 64*half + h_in, j, kx, h_out] = w_exp[c, h_in - h_out + PAD, kx]
    for c = 24*r + 4*g + 2*half + j, zero outside the |h_in - h_out| <= PAD band.
    """
    hi = np.arange(H)[:, None]
    ho = np.arange(H)[None, :]
    ky = hi - ho + PAD
    valid = (ky >= 0) & (ky < KS)
    kyc = np.clip(ky, 0, KS - 1)
    # w_exp[:, kyc, :] -> (C, h_in, h_out, kx); want B[c, h_in, kx, h_out]
    B = np.where(valid[None, :, None, :],
                 w_exp[:, kyc, :].transpose(0, 1, 3, 2), 0.0)
    B = np.ascontiguousarray(B, dtype=np.float32)
    bm = np.zeros((NCORES, NGROUPS, 128, 2, KS, H), dtype=np.float32)
    for r in range(NCORES):
        for g in range(NGROUPS):
            for j in range(2):
                for half in range(2):
                    c = 24 * r + 4 * g + 2 * half + j
                    bm[r, g, 64 * half:64 * half + 64, j] = B[c]
    return bm


def _emit_group(nc, pools, g, params):
    k_in, v_in, q_in, bmats, out_d, ones1, ones64 = params
    io_pool, xpool, spool, opool, psum_pool = pools

    # ---- staging loads (f32, unpadded) ----
    k_st = io_pool.tile([128, 2, NIMG, W], f32, tag="k_st")
    v_st = io_pool.tile([128, 2, NIMG, W], f32, tag="v_st")
    for j in range(2):
        for half in range(2):
            c = 4 * g + 2 * half + j
            p0 = 64 * half
            nc.sync.dma_start(k_st[p0:p0 + 64, j],
                              k_in[:, c, :].rearrange("b (h w) -> h b w", h=H))
            nc.sync.dma_start(v_st[p0:p0 + 64, j],
                              v_in[:, c, :].rearrange("b (h w) -> h b w", h=H))
    q_t = []
    for pair in range(2):
        qt = io_pool.tile([128, NIMG, W], f32, tag=f"q_t{pair}")
        for half in range(2):
            c = 4 * g + 2 * pair + half
            nc.sync.dma_start(qt[64 * half:64 * half + 64],
                              q_in[:, c, :].rearrange("b (h w) -> h b w", h=H))
        q_t.append(qt)

    b_t = xpool.tile([128, 2, KS, H], bf16, tag="b_t")
    nc.sync.dma_start(b_t[:], bmats[g])

    # ---- padded e/p tiles ----
    e_t = xpool.tile([128, 2, NIMG, WPAD], bf16, tag="e_t")
    p_t = xpool.tile([128, 2, NIMG, WPAD], bf16, tag="p_t")
    for t in (e_t, p_t):
        nc.gpsimd.memset(t[:, :, :, 0:PAD], 0.0)
        nc.gpsimd.memset(t[:, :, :, PAD + W:], 0.0)

    nc.scalar.activation(e_t[:, :, :, PAD:PAD + W], k_st[:],
                         mybir.ActivationFunctionType.Exp)
    nc.vector.tensor_mul(p_t[:, :, :, PAD:PAD + W], e_t[:, :, :, PAD:PAD + W], v_st[:])

    sq_t = []
    for pair in range(2):
        sq = spool.tile([128, NIMG, W], f32, tag=f"sq{pair}")
        nc.scalar.activation(sq[:], q_t[pair][:], mybir.ActivationFunctionType.Sigmoid)
        sq_t.append(sq)

    # ---- per-(channel, image) sums: w-reduce then ones-matmul over h ----
    red_t = spool.tile([128, 2, 2, NIMG], f32, tag="red_t")
    nc.vector.tensor_reduce(red_t[:, 0], e_t[:, :, :, PAD:PAD + W],
                            mybir.AxisListType.X, mybir.AluOpType.add)
    nc.vector.tensor_reduce(red_t[:, 1], p_t[:, :, :, PAD:PAD + W],
                            mybir.AxisListType.X, mybir.AluOpType.add)

    red_b = spool.tile([128, 2, 2, NIMG], bf16, tag="red_b")
    nc.scalar.copy(red_b.rearrange("p a b c -> p (a b c)"),
                   red_t.rearrange("p a b c -> p (a b c)"))
    s_ps = psum_pool.tile([128, 64], f32, tag="ps")
    for half in range(2):
        p0 = 64 * half
        nc.tensor.matmul(s_ps[p0:p0 + 64, :], ones1[p0:p0 + 64, :],
                         red_b[p0:p0 + 64].rearrange("p a b c -> p (a b c)"),
                         start=True, stop=True)
    s_sb = spool.tile([128, 2, 2, NIMG], bf16, tag="s_sb")
    nc.scalar.copy(s_sb.rearrange("p a b c -> p (a b c)"), s_ps[:])

    # ---- conv + bias accumulation: 2 tensors x 2 image phases x 4 quadrants ----
    xt = [e_t, p_t]
    for ph in range(2):
        i0 = 8 * ph
        psum_t = {}
        for t in range(2):
            for pair in range(2):
                ps = psum_pool.tile([128, 8, W], f32, tag="ps")
                psum_t[(t, pair)] = ps
                for half in range(2):
                    j = 2 * pair + half          # channel within group (c order)
                    jh, jj = divmod(j, 2)        # half/slot in s_sb layout
                    sp0 = 64 * jh
                    rhs = s_sb[sp0:sp0 + 64, t, jj, i0:i0 + 8].broadcast_to((64, 8, W))
                    nc.tensor.matmul(ps[64 * half:64 * half + 64],
                                     ones64[sp0:sp0 + 64, :], rhs,
                                     start=True, stop=False)
        for kx in range(KS):
            last = kx == KS - 1
            for t in range(2):
                for j in range(2):
                    for half in range(2):
                        cidx = 2 * half + j
                        pair, ohalf = divmod(cidx, 2)
                        ps = psum_t[(t, pair)]
                        p0 = 64 * half
                        nc.tensor.matmul(
                            ps[64 * ohalf:64 * ohalf + 64],
                            b_t[p0:p0 + 64, j, kx, :],
                            xt[t][p0:p0 + 64, j, i0:i0 + 8, kx:kx + W],
                            start=False, stop=last)

        # ---- eviction: out = sigmoid(q) * psum_p * recip(psum_e) ----
        for pair in range(2):
            den = opool.tile([128, 8, W], f32, tag=f"den{pair}")
            nc.vector.reciprocal_approx_fast(
                out=den.rearrange("p a b -> p (a b)"),
                in_=psum_t[(0, pair)][:].rearrange("p a b -> p (a b)"))
            o_t = opool.tile([128, 8, W], f32, tag=f"o_t{pair}")
            nc.vector.tensor_mul(o_t[:], psum_t[(1, pair)][:], den[:])
            nc.vector.tensor_mul(o_t[:], o_t[:], sq_t[pair][:, i0:i0 + 8, :])
            for half in range(2):
                c = 4 * g + 2 * pair + half
                nc.sync.dma_start(
                    out_d[i0:i0 + 8, c, :].rearrange("b (h w) -> h b w", h=H),
                    o_t[64 * half:64 * half + 64])


def _build_nc():
    nc = bacc.Bacc("TRN2", target_bir_lowering=False, debug=False,
                   num_devices=NCORES)
    k_in = nc.declare_dram_parameter("k_in", [NIMG, C_CORE, H * W], f32, isOutput=False)
    v_in = nc.declare_dram_parameter("v_in", [NIMG, C_CORE, H * W], f32, isOutput=False)
    q_in = nc.declare_dram_parameter("q_in", [NIMG, C_CORE, H * W], f32, isOutput=False)
    bmats = nc.declare_dram_parameter("bmats", [NGROUPS, 128, 2, KS, H], bf16, isOutput=False)
    out_d = nc.declare_dram_parameter("out", [NIMG, C_CORE, H * W], f32, isOutput=True)

    with tile.TileContext(nc) as tc:
        with ExitStack() as ctx:
            io_pool = ctx.enter_context(tc.tile_pool(name="io", bufs=2))
            xpool = ctx.enter_context(tc.tile_pool(name="x", bufs=2))
            spool = ctx.enter_context(tc.tile_pool(name="s", bufs=2))
            opool = ctx.enter_context(tc.tile_pool(name="o", bufs=2))
            cpool = ctx.enter_context(tc.tile_pool(name="c", bufs=1))
            psum_pool = ctx.enter_context(tc.tile_pool(name="psum", bufs=8, space="PSUM"))

            ones1 = cpool.tile([128, 64], bf16)
            nc.vector.memset(ones1[:], 1.0)
            ones64 = cpool.tile([128, 64], bf16)
            nc.vector.memset(ones64[:], 1.0 / 64.0)

            params = (k_in, v_in, q_in, bmats, out_d, ones1, ones64)
            pools = (io_pool, xpool, spool, opool, psum_pool)
            for g in range(NGROUPS):
                _emit_group(nc, pools, g, params)
    nc.compile()
    return nc


def _get_nc():
    if "nc" not in _CACHE:
        _CACHE["nc"] = _build_nc()
    return _CACHE["nc"]


def run(qkv, weights, trace=False):
    qkv = np.asarray(qkv, dtype=np.float32)
    weights = np.asarray(weights, dtype=np.float32)
    assert qkv.shape == (NIMG, 3 * C_FULL, H * W), qkv.shape
    assert weights.shape == (C_FULL, 1, KS, KS), weights.shape

    import ml_dtypes
    w_exp = np.expm1(weights[:, 0].astype(np.float64)).astype(np.float32)
    bm = _build_bmats(w_exp).astype(ml_dtypes.bfloat16)

    in_maps = []
    for r in range(NCORES):
        cs = slice(24 * r, 24 * r + 24)
        in_maps.append({
            "q_in": np.ascontiguousarray(qkv[:, cs, :]),
            "k_in": np.ascontiguousarray(qkv[:, 192 + 24 * r:192 + 24 * r + 24, :]),
            "v_in": np.ascontiguousarray(qkv[:, 384 + 24 * r:384 + 24 * r + 24, :]),
            "bmats": bm[r],
        })

    nc = _get_nc()
    res = run_bass_kernel_spmd(nc, in_maps, core_ids=list(range(NCORES)),
                               trace=trace)
    out = np.empty((NIMG, C_FULL, H * W), dtype=np.float32)
    for r in range(NCORES):
        out[:, 24 * r:24 * r + 24, :] = res.results[r]["out"]
    return out, res


def kernel(qkv, weights, H=None, W=None, **_unused):
    out, _ = run(qkv, weights)
    return out
